# revision 27
# baseline (speedup 1.0000x reference)
"""EnsembleGATDGFLayer Trainium2 kernel.

Data-parallel over batch: 64 graphs -> 8 NeuronCores, 8 graphs each.
All layout prep (transposes, weight folding) happens on host; the device
kernel is pure matmul + elementwise with zero on-chip transposes.

Math (per graph, N=512 nodes, D=256 feat, P=64 op-emb):
  dense = gate_d * (adj @ (X@W)) + X@W + b      (DenseGraphFlow)
  scores = X @ M @ X.T,  M = Wq.T diag(a_w) Wk / 16
  attn = softmax(leaky_relu(scores) * adj)
  gat = LN(gate_g * attn @ (X@Wv.T)) * g + b2   (GraphAttention)
  out = 0.5*(dense + gat)

Key tricks:
  - All matmuls fp32r (tf32; 4x fp32 rate at free-dim >= 256); host
    pre-rounds matmul operands RNE to tf32.
  - scores computed TRANSPOSED [l, e] so adj is only needed transposed
    (host-provided) and attn (=exp, unnormalized) feeds matmuls directly.
  - softmax 1/S normalization is per-row positive -> cancels inside the
    downstream LayerNorm (scale invariance): never computed at all.
  - sigmoid(x) == 0.5*tanh(x/2)+0.5: gates use ACT Tanh so every ACT func
    lives in one act-table set -> no table reloads; the +1/x0.5 factors fold
    into scalar_tensor_tensor consumers and pre-scaled weights.
  - rhs packing: [0.25*dgf_W | Wv.T] and [dgf_opW.T+b | gat_opW.T+b] halve
    the support/Whv and gate matmul counts.
  - rstd via Quake rsqrt + 2 Newton steps on DVE (no ACT Sqrt).
  - per-graph emission is software-pipelined: front(g+1) before back(g) so
    the PE always has independent matmuls while exp/leaky cook.
"""

import os

import numpy as np

B, N, DIN, DOUT, DOP = 64, 512, 256, 256, 64
NCORES = 8
G = B // NCORES
LN_EPS = 1e-5
NEG = 0.2
QMAGIC = 0x5F3759DF
USE_PRELU = os.environ.get("USE_PRELU", "1") != "0"

_BUILT = {}


def build_bass(g=G, mm_dt_name="float32r", apply_lng=False, use_prelu=None):
    """Build the per-core Bass module processing `g` graphs."""
    if use_prelu is None:
        use_prelu = USE_PRELU
    key = (g, mm_dt_name, apply_lng, use_prelu)
    if key in _BUILT:
        return _BUILT[key]

    import concourse.bass as bass
    import concourse.tile as tile
    from concourse import bacc, mybir

    f32 = mybir.dt.float32
    i32 = mybir.dt.int32
    fmm = getattr(mybir.dt, mm_dt_name)
    AF = mybir.ActivationFunctionType
    OP = mybir.AluOpType

    nc = bacc.Bacc(None, target_bir_lowering=False, debug=False)

    # -------- DRAM I/O --------
    xt_d = nc.dram_tensor("xt", [g, 2, 128, N], fmm, kind="ExternalInput")
    adjt_d = nc.dram_tensor("adjt", [g, 4, 128, N], fmm, kind="ExternalInput")
    eta_d = nc.dram_tensor("eta", [g, 65, N], fmm, kind="ExternalInput")
    wc_d = nc.dram_tensor("wcomb", [2, 128, 512], fmm, kind="ExternalInput")
    mq_d = nc.dram_tensor("mq", [2, 128, DIN], fmm, kind="ExternalInput")
    go_d = nc.dram_tensor("gcomb", [65, 512], fmm, kind="ExternalInput")
    ch_d = nc.dram_tensor("chalf", [1, DOUT], f32, kind="ExternalInput")
    lng_d = nc.dram_tensor("lngh", [1, DOUT], f32, kind="ExternalInput")
    out_d = nc.dram_tensor("out", [g, 4, 128, DOUT], f32, kind="ExternalOutput")

    mm = nc.tensor.matmul

    with tile.TileContext(nc) as tc:
        with (
            tc.tile_pool(name="const", bufs=1) as cpool,
            tc.tile_pool(name="work", bufs=2) as wpool,
            tc.tile_pool(name="ps1", bufs=6, space="PSUM") as ps1,
            tc.tile_pool(name="ps2", bufs=1, space="PSUM") as ps2,
        ):
            # -------- replicated params --------
            mq_t = cpool.tile([128, 2, DIN], fmm)
            wc_t = cpool.tile([128, 2, 512], fmm)
            go_t = cpool.tile([65, 512], fmm)
            cb_t = cpool.tile([128, DOUT], f32)
            if apply_lng:
                lng_t = cpool.tile([128, DOUT], f32)

            def load_consts():
                nc.sync.dma_start(out=mq_t[:],
                                  in_=mq_d[:].rearrange("c p m -> p c m"))
                nc.sync.dma_start(out=wc_t[:],
                                  in_=wc_d[:].rearrange("c p m -> p c m"))
                nc.sync.dma_start(out=go_t[:], in_=go_d[:])
                nc.sync.dma_start(out=cb_t[:],
                                  in_=ch_d[:].to_broadcast([128, DOUT]))
                if apply_lng:
                    nc.sync.dma_start(out=lng_t[:],
                                      in_=lng_d[:].to_broadcast([128, DOUT]))

            def loads(gi):
                xt = wpool.tile([128, 2, N], fmm, tag="xt", bufs=3)
                nc.sync.dma_start(out=xt[:],
                                  in_=xt_d[gi].rearrange("c p n -> p c n"))
                if gi == 0:
                    load_consts()
                adjt = wpool.tile([128, 4, N], fmm, tag="adjt", bufs=3)
                nc.sync.dma_start(out=adjt[:],
                                  in_=adjt_d[gi].rearrange("c p n -> p c n"))
                eta = wpool.tile([65, N], fmm, tag="eta", bufs=3)
                nc.sync.dma_start(out=eta[:], in_=eta_d[gi])
                return dict(xt=xt, adjt=adjt, eta=eta)

            def fA(gi, st):
                """yt + [support|Whv] projections for graph gi."""
                xt, adjt, eta = st["xt"], st["adjt"], st["eta"]

                # YT = M.T @ XT  [d', e]
                yt = wpool.tile([128, 2, N], fmm, tag="yt")
                for mc in range(2):
                    p = ps1.tile([128, N], f32, tag="ps1")
                    for kc in range(2):
                        mm(p[:], mq_t[:, kc, mc * 128:(mc + 1) * 128],
                           xt[:, kc, :], start=(kc == 0), stop=(kc == 1))
                    nc.scalar.copy(out=yt[:, mc, :], in_=p[:])

                # [0.25*support | Whv | 2.0 2.0] = X-projections, natural [l, m]
                comb = wpool.tile([128, 4, 516], fmm, tag="comb")
                nc.gpsimd.memset(comb[:, :, 512:514].bitcast(f32), 2.0)
                sup_c = wpool.tile([128, 4, DOUT], f32, tag="sup_c")
                cb_ap = cb_t[:]
                for lc in range(4):
                    p = ps1.tile([128, 512], f32, tag="ps1")
                    for kc in range(2):
                        mm(p[:], xt[:, kc, lc * 128:(lc + 1) * 128], wc_t[:, kc, :],
                           start=(kc == 0), stop=(kc == 1))
                    nc.scalar.copy(out=comb[:, lc, :512], in_=p[:])
                    # 0.5*support + c == 2*(0.25*support) + c
                    nc.vector.scalar_tensor_tensor(
                        out=sup_c[:, lc, :], in0=p[:, :DOUT], scalar=2.0,
                        in1=cb_ap, op0=OP.mult, op1=OP.add)

                st.update(yt=yt, comb=comb, sup_c=sup_c)
                return st

            def fB(gi, st):
                """scores + exp + gates for graph gi."""
                xt, adjt, eta, yt = st["xt"], st["adjt"], st["eta"], st["yt"]
                # scoresT [l, e] = X @ YT ; mask; leaky; exp
                al = wpool.tile([128, 4, N], f32, tag="al")
                for lc in range(4):
                    p = ps1.tile([128, N], f32, tag="ps1")
                    for kc in range(2):
                        mm(p[:], xt[:, kc, lc * 128:(lc + 1) * 128], yt[:, kc, :],
                           start=(kc == 0), stop=(kc == 1))
                    # adj >= 0 so leaky(s)*adj == leaky(s*adj): mask first
                    nc.vector.tensor_mul(out=al[:, lc, :], in0=p[:],
                                         in1=adjt[:, lc, :].bitcast(f32))
                ex = wpool.tile([128, 4, N], fmm, tag="ex")
                lk = wpool.tile([128, 4, N], f32, tag="lk")
                for h2 in range(2):
                    s = slice(h2 * 2, h2 * 2 + 2)
                    if use_prelu:
                        nc.scalar.activation(out=lk[:, s, :], in_=al[:, s, :],
                                             func=AF.Prelu, alpha=NEG)
                    else:
                        nc.vector.scalar_tensor_tensor(
                            out=lk[:, s, :], in0=al[:, s, :], scalar=NEG,
                            in1=al[:, s, :], op0=OP.mult, op1=OP.max)
                    nc.scalar.activation(out=ex[:, s, :], in_=lk[:, s, :],
                                         func=AF.Exp)

                # gates: [gate_d | gate_g] = sigmoid = 0.5*tanh(x/2)+0.5
                th = wpool.tile([128, 4, 512], f32, tag="th")
                for ec in range(4):
                    p = ps1.tile([128, 512], f32, tag="ps1")
                    mm(p[:], eta[:, ec * 128:(ec + 1) * 128], go_t[:],
                       start=True, stop=True)
                    nc.scalar.activation(out=th[:, ec, :], in_=p[:],
                                         func=AF.Tanh, scale=0.5)
                st.update(ex=ex, th=th)
                return st

            def bA(gi, st):
                """AS + dense for graph gi."""
                adjt, comb, sup_c, th = st["adjt"], st["comb"], st["sup_c"], st["th"]

                # AS = adjT.T @ (0.25*support), natural [e, m]
                as_ps = ps2.tile([128, 4, DOUT], f32, tag="ps2")
                for ec in range(4):
                    for lc in range(4):
                        mm(as_ps[:, ec, :], adjt[:, lc, ec * 128:(ec + 1) * 128],
                           comb[:, lc, :DOUT], start=(lc == 0), stop=(lc == 3))
                # dense = (tanh_d+1)*AS + (0.5*support + c)
                dn = wpool.tile([128, 4, DOUT], f32, tag="dn")
                nc.vector.scalar_tensor_tensor(
                    out=dn[:], in0=th[:, :, :DOUT], scalar=1.0, in1=as_ps[:],
                    op0=OP.add, op1=OP.mult)
                nc.gpsimd.tensor_add(out=dn[:], in0=dn[:], in1=sup_c[:])
                st.update(dn=dn)
                return st

            def bB(gi, st):
                """attn@Whv + LN + out for graph gi."""
                comb, ex, th, dn = st["comb"], st["ex"], st["th"], st["dn"]
                # v = (tanh_g+1) * (exp @ Whv)  (= 2S * gate_g*attn@Whv; the
                # positive per-row 2S factor cancels in the LayerNorm below,
                # except through eps -- corrected via the 2S column.)
                h = wpool.tile([128, 4, DOUT], f32, tag="h")
                scol = wpool.tile([128, 4, 1], f32, tag="scol")
                for ec in range(4):
                    p = ps1.tile([128, 258], f32, tag="ps1")
                    for lc in range(4):
                        mm(p[:], ex[:, lc, ec * 128:(ec + 1) * 128],
                           comb[:, lc, DOUT:DOUT + 258],
                           start=(lc == 0), stop=(lc == 3))
                    nc.vector.tensor_copy(out=scol[:, ec, :],
                                          in_=p[:, 256:257])
                    nc.vector.scalar_tensor_tensor(
                        out=h[:, ec, :], in0=th[:, ec, DOUT:], scalar=1.0,
                        in1=p[:, :DOUT], op0=OP.add, op1=OP.mult)

                # LayerNorm over m
                stats = wpool.tile([128, 4, 6], f32, tag="stats")
                mv = wpool.tile([128, 4, 2], f32, tag="mv")
                for ec in range(4):
                    nc.vector.bn_stats(out=stats[:, ec, :], in_=h[:, ec, :])
                    nc.vector.bn_aggr(out=mv[:, ec, :], in_=stats[:, ec, :])
                # rstd (or rstd/2) via Quake rsqrt + 2 Newton steps (DVE only)
                # w = sc0*(var_v + eps*(2S)^2); rsqrt(w) absorbs the 2S scale
                w = wpool.tile([128, 4, 1], f32, tag="w")
                s2 = wpool.tile([128, 4, 1], f32, tag="s2")
                sc0 = 1.0 if apply_lng else 4.0
                nc.vector.tensor_mul(out=s2[:], in0=scol[:], in1=scol[:])
                nc.vector.tensor_scalar(
                    out=w[:], in0=mv[:, :, 1:2], scalar1=sc0,
                    scalar2=None, op0=OP.mult)
                nc.vector.scalar_tensor_tensor(
                    out=w[:], in0=s2[:], scalar=sc0 * LN_EPS, in1=w[:],
                    op0=OP.mult, op1=OP.add)
                yq = wpool.tile([128, 4, 1], f32, tag="yq")
                tq = wpool.tile([128, 4, 1], i32, tag="tq")
                nc.vector.tensor_scalar(
                    out=tq[:], in0=w[:].bitcast(i32), scalar1=1,
                    scalar2=None, op0=OP.arith_shift_right)
                nc.vector.tensor_scalar(
                    out=yq[:].bitcast(i32), in0=tq[:], scalar1=QMAGIC,
                    scalar2=-1, op0=OP.subtract, op1=OP.mult)
                aq = wpool.tile([128, 4, 1], f32, tag="aq")
                for _ in range(2):
                    nc.vector.tensor_mul(out=aq[:], in0=yq[:], in1=yq[:])
                    nc.vector.scalar_tensor_tensor(
                        out=aq[:], in0=aq[:], scalar=-0.5, in1=w[:],
                        op0=OP.mult, op1=OP.mult)
                    nc.vector.scalar_tensor_tensor(
                        out=yq[:], in0=aq[:], scalar=1.5, in1=yq[:],
                        op0=OP.add, op1=OP.mult)
                # nb = -mu * rstd
                nb = wpool.tile([128, 4, 1], f32, tag="nb")
                nc.vector.scalar_tensor_tensor(
                    out=nb[:], in0=mv[:, :, 0:1], scalar=-1.0, in1=yq[:],
                    op0=OP.mult, op1=OP.mult)
                t = wpool.tile([128, 4, DOUT], f32, tag="t")
                for ec in range(4):
                    nc.scalar.activation(out=t[:, ec, :], in_=h[:, ec, :],
                                         func=AF.Identity, bias=nb[:, ec, :],
                                         scale=yq[:, ec, :])
                fin = wpool.tile([128, 4, DOUT], f32, tag="fin")
                if apply_lng:
                    for ec in range(4):
                        nc.gpsimd.tensor_mul(out=t[:, ec, :], in0=t[:, ec, :],
                                             in1=lng_t[:])
                nc.gpsimd.tensor_add(out=fin[:], in0=t[:], in1=dn[:])
                nc.sync.dma_start(out=out_d[gi].rearrange("c p m -> p c m"),
                                  in_=fin[:])

            # PE warmup: keep the HAM activity monitor busy while the first
            # graph's DMAs land so real matmuls start at full clock.
            wup = cpool.tile([128, N], fmm)
            nc.gpsimd.memset(wup[:].bitcast(f32), 0.25)
            for _ in range(44):
                pw = ps1.tile([128, N], f32, tag="ps1")
                mm(pw[:], wup[:, :128], wup[:], start=True, stop=True)

            # software pipeline, quarter-phase interleave:
            #   fA(g+1) -> bA(g) -> bB(g) -> fB(g+1)
            sts = {0: loads(0)}
            if g > 1:
                sts[1] = loads(1)
            fA(0, sts[0])
            fB(0, sts[0])
            for gi in range(1, g):
                if gi + 1 < g:
                    sts[gi + 1] = loads(gi + 1)
                fA(gi, sts[gi])
                bA(gi - 1, sts[gi - 1])
                bB(gi - 1, sts[gi - 1])
                del sts[gi - 1]
                fB(gi, sts[gi])
            bA(g - 1, sts[g - 1])
            bB(g - 1, sts[g - 1])

    nc.compile()
    _BUILT[key] = nc
    return nc


def tf32_round(a):
    """Round-to-nearest-even fp32 -> tf32 (10-bit mantissa) == fp32r."""
    u = np.ascontiguousarray(a, np.float32).view(np.uint32)
    u = (u + np.uint32(0x0FFF) + ((u >> np.uint32(13)) & np.uint32(1))) \
        & np.uint32(0xFFFFE000)
    return u.view(np.float32)


def prep_host(inputs, adj, op_emb, dgf_W, dgf_b, dgf_opW, dgf_opb,
              Wk, Wv, Wq, a_w, gat_opW, gat_opb, ln_g, ln_b):
    """Fold params + lay out per-graph tensors for the device kernel."""
    f = np.float32
    x = np.asarray(inputs, f)
    adj = np.asarray(adj, f)
    ope = np.asarray(op_emb, f)
    nb = x.shape[0]

    xt = np.ascontiguousarray(x.transpose(0, 2, 1)).reshape(nb, 2, 128, N)
    adjt = np.ascontiguousarray(adj.transpose(0, 2, 1)).reshape(nb, 4, 128, N)
    et = np.ascontiguousarray(ope.transpose(0, 2, 1))  # [nb, 64, N]
    eta = np.concatenate([et, np.ones((nb, 1, N), f)], axis=1)  # [nb, 65, N]

    wcomb = np.ascontiguousarray(np.concatenate(
        [0.25 * np.asarray(dgf_W, f), np.asarray(Wv, f).T],
        axis=1)).reshape(2, 128, 512)
    mq = np.ascontiguousarray(
        (np.asarray(Wq, f).T * np.asarray(a_w, f)[None, :]) @ np.asarray(Wk, f)
        / np.sqrt(np.float32(DOUT))).reshape(2, 128, DIN)
    gcomb = np.ascontiguousarray(np.concatenate([
        np.concatenate([np.asarray(dgf_opW, f).T,
                        np.asarray(dgf_opb, f)[None, :]], 0),
        np.concatenate([np.asarray(gat_opW, f).T,
                        np.asarray(gat_opb, f)[None, :]], 0)], axis=1))
    ch = np.ascontiguousarray(
        (0.5 * (np.asarray(dgf_b, f) + np.asarray(ln_b, f))).reshape(1, DOUT))
    lng = np.ascontiguousarray((0.5 * np.asarray(ln_g, f)).reshape(1, DOUT))
    apply_lng = not (np.all(np.asarray(ln_g, f) == 1.0))
    hp = dict(xt=xt, adjt=adjt, eta=eta, wcomb=wcomb, mq=mq, gcomb=gcomb,
              chalf=ch, lngh=lng)
    if MM_DT == "float32r":
        # matmul-feeding tensors must carry fp32r(=tf32)-rounded values
        for k in ("xt", "adjt", "eta", "wcomb", "mq", "gcomb"):
            hp[k] = tf32_round(hp[k])
    return hp, apply_lng


MM_DT = "float32r"


def run(hp, apply_lng, mm_dt=None, trace=False, **kw):
    from concourse.bass_utils import run_bass_kernel_spmd

    nc = build_bass(G, mm_dt or MM_DT, apply_lng)
    in_maps = []
    for c in range(NCORES):
        sl = slice(c * G, (c + 1) * G)
        m = {k: (v[sl] if k in ("xt", "adjt", "eta") else v)
             for k, v in hp.items()}
        in_maps.append(m)
    res = run_bass_kernel_spmd(nc, in_maps, core_ids=list(range(NCORES)),
                               trace=trace, **kw)
    out = np.concatenate(
        [r["out"].reshape(G, N, DOUT) for r in res.results], axis=0)
    return np.ascontiguousarray(out), res


def kernel(**inputs) -> np.ndarray:
    hp, apply_lng = prep_host(**inputs)
    out, _ = run(hp, apply_lng)
    return out
